# revision 44
# baseline (speedup 1.0000x reference)
"""Trainium2 Bass kernel for nn_JointRelationModule (self-contained).

Math (per person p; softmax is segment-softmax over persons within an imgid
group, elementwise over the (K,K) score entries):
    q = Wq x + bq ; k = Wk x + bk ; v = Wv x + bv      (1x1 conv over K=17)
    S_p = q_p k_p^T / 64
    attn = segment-softmax over persons
    out = relu(attn_p @ v_p + x_p)

Device formulation (heavy ops bf16 on the PE, block-column layouts):
  - Stack BD=7 persons as [119, hw]. Per stack: G = x x^T via PE transpose +
    accumulating matmuls (bf16, f32 PSUM).
  - scores^T in block-column layout [119, 17] via a masked-Gram matmul chain
    (block-diag mask kills cross-person terms), so no gather/scatter DMAs.
  - Segment softmax via per-stack selector matmuls into group-slot tiles,
    reciprocal, selector-transpose broadcast back; all partition-aligned.
  - Output: B = blockdiag((attn Wv)^T) + I with an av row appended; the
    residual and v-bias ride along x_aug (all-ones row), so each output chunk
    is one matmul + one relu. Stored bf16, host upcasts.

Data movement: x and y live in a partition-major layout [120, S*hw] so a
multi-stack tile is one DMA with 16KB-contiguous per-partition descriptors
(per-queue DMA throughput here is descriptor-rate-limited); every load/store
is split across the three DMA-capable queues (sync/gpsimd/scalar) by
partition range. Loads are emitted just-in-time with the compute emission.

Sharding: data-parallel over persons at imgid group boundaries (8 cores),
weights replicated. Host casts x to bf16 (halves load bytes); output comes
back bf16 (halves store bytes). Tolerance 2e-2; measured error ~5e-3.
"""

import math
import sys

import numpy as np

K = 17
HW = 4096  # 64*64
P_TOTAL = 512
N_CORES = 8
NORM = 64.0
BD = 7          # persons per stack
BDK = BD * K    # 119
O_CH = 512      # output chunk cols (one PSUM bank of f32)

_cache: dict = {}


def _ensure_path():
    try:
        import concourse.bass  # noqa: F401
    except ImportError:
        for p in ("/opt/trn_rl_repo", "/root/.axon_site/_ro/trn_rl_repo"):
            if p not in sys.path:
                sys.path.insert(0, p)
        import concourse.bass  # noqa: F401


def _build(P_pad: int, T: int, have_bias: bool, used: tuple):
    """Builds + compiles the per-core SPMD Bass program."""
    _ensure_path()
    import concourse.bacc as bacc
    import concourse.mybir as mybir
    import concourse.tile as tile

    f32 = mybir.dt.float32
    bf16 = mybir.dt.bfloat16
    Exp = mybir.ActivationFunctionType.Exp
    Relu = mybir.ActivationFunctionType.Relu

    S = P_pad // BD
    assert P_pad % BD == 0
    U = len(used)

    nc = bacc.Bacc(
        "TRN2",
        target_bir_lowering=False,
        debug=False,
        enable_asserts=False,
        num_devices=N_CORES,
    )

    x_d = nc.dram_tensor("x", [BDK + 1, S * HW], bf16, kind="ExternalInput")
    wq_d = nc.dram_tensor("wq_col", [BDK, K], f32, kind="ExternalInput")
    wk_d = nc.dram_tensor("wkt_bd", [BDK, BDK], f32, kind="ExternalInput")
    wv_d = nc.dram_tensor("wv_aug", [BDK, BDK + 1], bf16, kind="ExternalInput")
    id_d = nc.dram_tensor("id119", [BDK, BDK], bf16, kind="ExternalInput")
    ia_d = nc.dram_tensor("iaug", [BDK + 1, BDK], f32, kind="ExternalInput")
    mk_d = nc.dram_tensor("bdmask", [BDK, BDK], f32, kind="ExternalInput")
    mkb_d = nc.dram_tensor("bdmaskb", [BDK, BDK], bf16, kind="ExternalInput")
    sel_d = nc.dram_tensor("sel", [BDK, U * BDK], bf16, kind="ExternalInput")
    selt_d = nc.dram_tensor("selT", [BDK, U * BDK], bf16,
                            kind="ExternalInput")
    if have_bias:
        corr_d = nc.dram_tensor("corr_col", [BDK, K * S], f32,
                                kind="ExternalInput")
    y_d = nc.dram_tensor("y", [BDK, S * HW], bf16, kind="ExternalOutput")

    with tile.TileContext(nc) as tc:
        with (
            nc.allow_low_precision(reason="bf16 softmax ok at 2e-2 tol"),
            tc.tile_pool(name="xpool", bufs=1) as xpool,
            tc.tile_pool(name="cpool", bufs=1) as cpool,
            tc.tile_pool(name="wpool", bufs=2) as wpool,
            tc.tile_pool(name="opool", bufs=2) as opool,
            tc.tile_pool(name="pp", bufs=2, space="PSUM") as pp,
        ):
            queues = (nc.sync, nc.gpsimd, nc.scalar)
            PSPLIT = (slice(0, 40), slice(40, 80), slice(80, BDK + 1))
            PSPLIT_Y = (slice(0, 40), slice(40, 80), slice(80, BDK))

            # --- tiny phase-A constants first (id_t gates every transpose) ---
            id_t = cpool.tile([BDK, BDK], bf16, name="id_t", tag="id")
            mk_t = cpool.tile([BDK, BDK], f32, name="mk_t", tag="mk")
            wq_t = cpool.tile([BDK, K], f32, name="wq_t", tag="wq")
            wk_t = cpool.tile([BDK, BDK], f32, name="wk_t", tag="wk")
            nc.sync.dma_start(id_t[:], id_d.ap())
            nc.gpsimd.dma_start(mk_t[:], mk_d.ap())
            nc.sync.dma_start(wq_t[:], wq_d.ap())
            nc.gpsimd.dma_start(wk_t[:], wk_d.ap())

            x_tiles = []  # per stack

            def load_xtile(s):
                xt_ = xpool.tile([BDK + 1, HW], bf16, name=f"xp{s}",
                                 tag=f"xp{s}")
                csl = slice(s * HW, (s + 1) * HW)
                for qi, psl in enumerate(PSPLIT):
                    queues[qi].dma_start(xt_[psl, :], x_d.ap()[psl, csl])
                x_tiles.append(xt_)

            load_xtile(0)
            if have_bias:
                corr_t = cpool.tile([BDK, K * S], f32, name="corr_t",
                                    tag="corr")
                nc.scalar.dma_start(corr_t[:], corr_d.ap())

            # bulkier constants: tiles declared now, DMAs emitted mid-phase-A
            # (the framework coalesces DMA waits into a cumulative counter, so
            # anything emitted before the first transpose delays it)
            wv_t = cpool.tile([BDK, BDK + 1], bf16, name="wv_t", tag="wv")
            ia_t = cpool.tile([BDK + 1, BDK], f32, name="ia_t", tag="ia")
            sel_t = cpool.tile([BDK, U * BDK], bf16, name="sel_t", tag="sel")
            selt_t = cpool.tile([BDK, U * BDK], bf16, name="selt_t",
                                tag="selt")
            mkb_t = cpool.tile([BDK, BDK], bf16, name="mkb_t", tag="mkb")

            def emit_const_dmas():
                nc.sync.dma_start(wv_t[:], wv_d.ap())
                nc.gpsimd.dma_start(ia_t[:], ia_d.ap())
                nc.scalar.dma_start(mkb_t[:], mkb_d.ap())
                nc.sync.dma_start(sel_t[:], sel_d.ap())
                nc.gpsimd.dma_start(selt_t[:], selt_d.ap())

            exp_all = cpool.tile([BDK, K * S], bf16, name="exp_all", tag="exp")
            inv_t = cpool.tile([BDK, K * T], bf16, name="inv_t", tag="inv")

            # --- phase A: transpose -> gram -> scores^T -> exp, skewed ---
            # PSUM tags (8 banks): big=tp/o_ps x4, gsb=g/seg/b x2, tiny x2
            G_CH = 1024          # x cols per transpose group
            n_grp = HW // G_CH   # 4 groups per stack
            TC = BDK + 1         # 120: chunk col stride (4B-aligned in PSUM)
            state = {"ncopy": 0, "loaded": 1}
            g_tiles = {}

            def emit_transposes(s, gi):
                xt_ = x_tiles[s]
                tp = pp.tile([128, 8 * TC], bf16, name="tp", tag="big",
                             bufs=4)
                for c8 in range(8):
                    col = G_CH * gi + 128 * c8
                    nc.tensor.transpose(
                        tp[:, TC * c8:TC * c8 + BDK],
                        xt_[0:BDK, col:col + 128], id_t[:],
                    )
                xt = wpool.tile([128, 8 * TC], bf16, name="xt", tag="xt",
                                bufs=4)
                if state["ncopy"] % 2 == 0:
                    nc.vector.tensor_copy(xt[:], tp[:])
                else:
                    nc.scalar.copy(xt[:], tp[:])
                state["ncopy"] += 1
                return xt

            def emit_gram(s, gi, xt):
                if s not in g_tiles:
                    g_tiles[s] = pp.tile([BDK + 1, BDK], f32, name=f"g{s}",
                                         tag="gsb", bufs=2)
                g_ps = g_tiles[s]
                for c8 in range(8):
                    nc.tensor.matmul(
                        g_ps[0:BDK, :], xt[:, TC * c8:TC * c8 + BDK],
                        xt[:, TC * c8:TC * c8 + BDK],
                        start=(gi == 0 and c8 == 0),
                        stop=(gi == n_grp - 1 and c8 == 7),
                    )
                if gi == n_grp - 1:
                    emit_tiny_chain(s)

            def emit_tiny_chain(s):
                g_sb = wpool.tile([BDK, BDK], f32, name="g_sb", tag="g_sb",
                                  bufs=2)
                nc.vector.tensor_mul(g_sb[:], g_tiles[s][0:BDK, :], mk_t[:])
                m1_ps = pp.tile([BDK, K], f32, name="m1", tag="tiny", bufs=2)
                nc.tensor.matmul(m1_ps[:], g_sb[:], wq_t[:], start=True,
                                 stop=True)
                m1_sb = wpool.tile([BDK, K], f32, name="m1_sb", tag="m1_sb",
                                   bufs=2)
                nc.scalar.copy(m1_sb[:], m1_ps[:])
                st_ps = pp.tile([BDK, K], f32, name="st", tag="tiny", bufs=2)
                nc.tensor.matmul(st_ps[:], wk_t[:], m1_sb[:], start=True,
                                 stop=True)
                esl = slice(K * s, K * (s + 1))
                if have_bias:
                    eb_sb = wpool.tile([BDK, K], f32, name="eb_sb", tag="eb")
                    nc.vector.tensor_add(eb_sb[:], st_ps[:], corr_t[:, esl])
                    nc.scalar.activation(exp_all[:, esl], eb_sb[:], Exp)
                else:
                    nc.scalar.activation(exp_all[:, esl], st_ps[:], Exp)

            pend = []
            for s in range(S):
                if s == 2:
                    emit_const_dmas()
                for gi in range(n_grp):
                    pend.append((s, gi, emit_transposes(s, gi)))
                    # just-in-time prefetch, interleaved with compute emission
                    while state["loaded"] < min(s + 3, S):
                        load_xtile(state["loaded"])
                        state["loaded"] += 1
                    if len(pend) > 2:
                        ps, pgi, xt = pend.pop(0)
                        emit_gram(ps, pgi, xt)
            for ps, pgi, xt in pend:
                emit_gram(ps, pgi, xt)

            # --- phase C: segment sums -> reciprocal ---
            seg_tiles = []
            for t in range(T):
                idxs = [i for i, (ss, tt) in enumerate(used) if tt == t]
                seg_ps = pp.tile([BDK + 1, BDK], f32, name=f"seg{t}",
                                 tag="gsb", bufs=2)
                for n, i in enumerate(idxs):
                    s = used[i][0]
                    nc.tensor.matmul(
                        seg_ps[0:BDK, 0:K],
                        sel_t[:, BDK * i:BDK * (i + 1)],
                        exp_all[:, K * s:K * (s + 1)],
                        start=(n == 0), stop=(n == len(idxs) - 1),
                    )
                seg_tiles.append(seg_ps)
            for t in range(T):
                seg_sb = wpool.tile([BDK, K], f32, name="seg_sb",
                                    tag="seg_sb")
                nc.vector.tensor_scalar_max(seg_sb[:],
                                            seg_tiles[t][0:BDK, 0:K], 1e-30)
                nc.vector.reciprocal(inv_t[:, K * t:K * (t + 1)], seg_sb[:])

            # --- phase D: pipelined per stack ---
            nrelu = 0
            attn_tiles = {}

            def emit_attn_chain(s):
                idxs = [i for i, (ss, tt) in enumerate(used) if ss == s]
                invb_ps = pp.tile([BDK, K], f32, name="invb", tag="tiny",
                                  bufs=2)
                for n, i in enumerate(idxs):
                    t = used[i][1]
                    nc.tensor.matmul(
                        invb_ps[:],
                        selt_t[:, BDK * i:BDK * (i + 1)],
                        inv_t[:, K * t:K * (t + 1)],
                        start=(n == 0), stop=(n == len(idxs) - 1),
                    )
                attn_sb = wpool.tile([BDK, K], bf16, name="attn_sb",
                                     tag="attn_c", bufs=2)
                nc.vector.tensor_mul(attn_sb[:], exp_all[:, K * s:K * (s + 1)],
                                     invb_ps[:])
                attn_bd = wpool.tile([BDK, BDK], bf16, name="attn_bd",
                                     tag="attn", bufs=2)
                for j in range(BD):
                    jsl = slice(K * j, K * (j + 1))
                    eng = nc.gpsimd if j % 2 == 0 else nc.vector
                    eng.tensor_mul(attn_bd[:, jsl], attn_sb[:], mkb_t[:, jsl])
                attn_tiles[s] = attn_bd

            emit_attn_chain(0)
            osb = None
            for s in range(S):
                b_ps = pp.tile([BDK + 1, BDK], f32, name="b_ps", tag="gsb",
                               bufs=2)
                nc.tensor.matmul(b_ps[:], wv_t[:], attn_tiles.pop(s)[:],
                                 start=True, stop=True)
                b_sb = wpool.tile([BDK + 1, BDK], bf16, name="b_sb", tag="B",
                                  bufs=2)
                nc.vector.tensor_add(b_sb[:], b_ps[:], ia_t[:])
                if s + 1 < S:
                    emit_attn_chain(s + 1)  # overlaps this stack's matmuls

                osb = opool.tile([BDK, HW], bf16, name="osb", tag="osb",
                                 bufs=3)
                xt_ = x_tiles[s]
                for oc in range(HW // O_CH):
                    sl = slice(O_CH * oc, O_CH * (oc + 1))
                    o_ps = pp.tile([BDK, O_CH], f32, name="o_ps", tag="big",
                                   bufs=4)
                    nc.tensor.matmul(o_ps[:], b_sb[:], xt_[:, sl],
                                     start=True, stop=True)
                    # drain each chunk with ACT+DVE halves concurrently: the
                    # PSUM buffer frees ~2x sooner, so the PE stalls less
                    h = O_CH // 2
                    sla = slice(sl.start, sl.start + h)
                    slb = slice(sl.start + h, sl.stop)
                    if nrelu % 2 == 0:
                        nc.scalar.activation(osb[:, sla], o_ps[:, 0:h], Relu)
                        nc.vector.tensor_scalar_max(osb[:, slb],
                                                    o_ps[:, h:O_CH], 0.0)
                    else:
                        nc.vector.tensor_scalar_max(osb[:, sla],
                                                    o_ps[:, 0:h], 0.0)
                        nc.scalar.activation(osb[:, slb], o_ps[:, h:O_CH],
                                             Relu)
                    nrelu += 1
                queues[s % 3].dma_start(
                    y_d.ap()[:, s * HW:(s + 1) * HW], osb[:])

    nc.compile()
    return nc


def _get_compiled(P_pad: int, T: int, have_bias: bool, used: tuple):
    key = (P_pad, T, have_bias, used)
    if key not in _cache:
        _cache[key] = _build(P_pad, T, have_bias, used)
    return _cache[key]


def _bd7(m: np.ndarray) -> np.ndarray:
    out = np.zeros((BDK, BDK), dtype=np.float32)
    for j in range(BD):
        out[K * j:K * (j + 1), K * j:K * (j + 1)] = m
    return out


def _plan(ids: np.ndarray):
    """Split persons into N_CORES contiguous chunks at imgid boundaries."""
    change = np.flatnonzero(np.diff(ids)) + 1
    allb = np.concatenate([[0], change, [P_TOTAL]]).astype(np.int64)
    bounds = [0]
    for ci in range(1, N_CORES):
        target = P_TOTAL * ci / N_CORES
        cand = allb[allb > bounds[-1]]
        if len(cand) == 0:
            bounds.append(bounds[-1])
        else:
            bounds.append(int(cand[np.argmin(np.abs(cand - target))]))
    bounds.append(P_TOTAL)
    sizes = np.diff(bounds)
    P_max = int(sizes.max())
    P_pad = max(BD, BD * math.ceil(P_max / BD))
    ng_max = 1
    for ci in range(N_CORES):
        a, b = bounds[ci], bounds[ci + 1]
        ng_max = max(ng_max, len(np.unique(ids[a:b])) + 1)
    T = math.ceil(ng_max / BD)
    return bounds, P_pad, T


def _prepare(inputs: dict):
    import ml_dtypes
    nbf16 = ml_dtypes.bfloat16

    x = np.asarray(inputs["kpt_feat"], dtype=np.float32).reshape(
        P_TOTAL, K, HW)
    ids = np.asarray(inputs["imgid"]).astype(np.int64)
    Wq = np.asarray(inputs["Wq"], np.float32)
    Wk = np.asarray(inputs["Wk"], np.float32)
    Wv = np.asarray(inputs["Wv"], np.float32)
    bq = np.asarray(inputs["bq"], np.float32)
    bk = np.asarray(inputs["bk"], np.float32)
    bv = np.asarray(inputs["bv"], np.float32)

    bounds, P_pad, T = _plan(ids)
    S = P_pad // BD

    wq_col = np.zeros((BDK, K), np.float32)
    for j in range(BD):
        wq_col[K * j:K * (j + 1), :] = Wq.T / NORM
    wkt_bd = _bd7(Wk.T.astype(np.float32))
    wv_aug = np.zeros((BDK, BDK + 1), np.float32)
    wv_aug[:, :BDK] = _bd7(Wv)
    for j in range(BD):
        wv_aug[K * j:K * (j + 1), BDK] = bv
    wv_aug = wv_aug.astype(nbf16)
    id119 = np.eye(BDK, dtype=np.float32).astype(nbf16)
    iaug = np.zeros((BDK + 1, BDK), np.float32)
    iaug[:BDK, :BDK] = np.eye(BDK, dtype=np.float32)
    bdmask = _bd7(np.ones((K, K), np.float32))

    have_bias = bool(np.any(bq) or np.any(bk))
    if have_bias:
        xsum = x.sum(axis=2)
        qx = xsum @ Wq.T
        kx = xsum @ Wk.T
        corr_all = (bk[None, :, None] * qx[:, None, :]
                    + bq[None, None, :] * kx[:, :, None]
                    + HW * (bq[None, None, :] * bk[None, :, None])) / NORM
        corr_all = corr_all.astype(np.float32)  # [P, m, i]
    else:
        corr_all = None

    xb = x.astype(nbf16)

    # selector tensors per core + union of nonzero (s, t) pairs
    eye = np.eye(K, dtype=np.float32)
    sels = []
    used_set = set()
    for ci in range(N_CORES):
        a, b = bounds[ci], bounds[ci + 1]
        pc = b - a
        slots = np.full((P_pad,), 7 * T - 1, np.int64)
        if pc:
            _, lg = np.unique(ids[a:b], return_inverse=True)
            slots[:pc] = lg
        sel = np.zeros((S, T, BDK, BDK), np.float32)
        for s in range(S):
            for j in range(BD):
                g = slots[BD * s + j]
                t, lgi = divmod(g, BD)
                sel[s, t, K * j:K * (j + 1), K * lgi:K * (lgi + 1)] = eye
                used_set.add((s, t))
        sels.append(sel)
    used = tuple(sorted(used_set))

    in_maps = []
    for ci in range(N_CORES):
        a, b = bounds[ci], bounds[ci + 1]
        pc = b - a
        # partition-major x: [120, S*HW]; row 119 = ones (residual fold)
        rows = np.zeros((P_pad * K, HW), dtype=nbf16)
        if pc:
            rows[:pc * K] = xb[a:b].reshape(pc * K, HW)
        arr3 = np.zeros((S, BDK + 1, HW), dtype=nbf16)
        arr3[:, :BDK] = rows.reshape(S, BDK, HW)
        arr3[:, BDK] = 1.0
        xs = np.ascontiguousarray(
            arr3.transpose(1, 0, 2).reshape(BDK + 1, S * HW))
        sel = sels[ci]
        su = np.stack([sel[s, t] for (s, t) in used])  # [U, 119, 119]
        sel_pack = su.transpose(1, 0, 2).reshape(BDK, len(used) * BDK)
        selt_pack = su.transpose(2, 0, 1).reshape(BDK, len(used) * BDK)
        m = {
            "x": xs,
            "wq_col": wq_col,
            "wkt_bd": wkt_bd,
            "wv_aug": wv_aug,
            "id119": id119,
            "iaug": iaug,
            "bdmask": bdmask,
            "bdmaskb": bdmask.astype(nbf16),
            "sel": np.ascontiguousarray(sel_pack).astype(nbf16),
            "selT": np.ascontiguousarray(selt_pack).astype(nbf16),
        }
        if have_bias:
            corr_col = np.zeros((BDK, K * S), np.float32)
            if pc:
                cpad = np.zeros((P_pad, K, K), np.float32)
                cpad[:pc] = corr_all[a:b]
                for s in range(S):
                    for j in range(BD):
                        corr_col[K * j:K * (j + 1), K * s:K * (s + 1)] = \
                            cpad[BD * s + j]
            m["corr_col"] = corr_col
        in_maps.append(m)
    return in_maps, bounds, P_pad, T, have_bias, used


def _gather(results, bounds, P_pad):
    S = P_pad // BD
    out = np.empty((P_TOTAL, K, 64, 64), dtype=np.float32)
    for ci in range(N_CORES):
        a, b = bounds[ci], bounds[ci + 1]
        pc = b - a
        if pc:
            y = np.asarray(results[ci]["y"], dtype=np.float32)  # [119, S*HW]
            y = y.reshape(BDK, S, HW).transpose(1, 0, 2).reshape(
                P_pad, K, 64, 64)
            out[a:b] = y[:pc]
    return out


def _run(inputs: dict, trace: bool = False):
    _ensure_path()
    from concourse.bass_utils import run_bass_kernel_spmd

    in_maps, bounds, P_pad, T, have_bias, used = _prepare(inputs)
    nc = _get_compiled(P_pad, T, have_bias, used)
    res = run_bass_kernel_spmd(nc, in_maps, list(range(N_CORES)), trace=trace)
    return _gather(res.results, bounds, P_pad), res


def kernel(**inputs) -> np.ndarray:
    out, _ = _run(inputs, trace=False)
    return out
